# revision 14
# baseline (speedup 1.0000x reference)
"""Trainium2 Bass kernel for CustomGPT2MultiHeadAttention (B=4, S=1024, SI=512,
D=1024, 16 heads), sharded over 8 NeuronCores.

Sharding: core c handles (batch b = c//2, head-group hg = c%2 of 8 heads).
Tensor-parallel on heads for QKV/attention; after the (per-core partial)
output projection, a pairwise ReduceScatter over {2b, 2b+1} produces disjoint
sequence halves of the final output, which the host concatenates.

All inputs are cast to bf16 on the host (the device consumed bf16 for every
matmul anyway), halving input DMA bytes and removing all on-device input
casts.  Engine budget per core (cost-model):
  PE   ~150us  704 matmuls x N=512 (projections 256, scores 192, xmm 192,
               y 64) -- MM-count optimal: scores are output-element-bound,
               the xmm's M=65 (64 V dims + softmax-denominator ones row)
               cannot pack two heads into M=128 because each head streams a
               different pT.
  ACT  ~100us  96 exp tiles [128,1024] (the only op class on ACT).
  DVE   ~90us  mask multiplies (bf16 2x), qT/vA PSUM->SBUF copies,
               reciprocals, normalize muls.
  Pool  ~30us  kT copies, partition broadcasts, part of the y casts.
The attention phase alone is ACT-bound (2 exps = 2.1us per key-tile vs
1.7us of PE work), so projection matmuls for later head-pairs are emitted
*between* attention key-tiles, through the same 2-slot PSUM ring the score
tiles use: the PE fills its exp-wait gaps with projection work and stays
dense for the whole kernel.

Device-side math per core:
  qT[o,s]  = w_q[hg] @ hidden[b]^T          (bf16 matmuls, f32 PSUM accum)
  kT[o,k'] = w_k[hg] @ hidden[b]^T  ++  u_k[hg] @ image[b]^T
  v[k',o]  = (hidden[b] ++ image[b]) @ w_v/u_v[hg]^T   (natural layout)
  per head: scoresT[k',q] = kT^T-slice . qT-slice  (K=64 contraction; head
            pairs use disjoint PE row groups 0/64 -> concurrent on silicon)
  pT = exp(scoresT/8) * maskT  (exp on ACT; no max-subtraction needed:
                                scores ~ N(0,1), exp range is safe in f32)
  per head: xT_aug[65,q] += [v | 1]^T . pT   (row 64 = masked softmax sums)
            xT[d,q] = xT_aug[0:64] * (1/sums)  (partition-broadcast recip)
  y_part[s,o] = xT^T . w_o^T[d-slice]          (partial over d)
  ReduceScatter(add) over the core pair -> y half [512, 1024] per core.
"""

import numpy as np
import ml_dtypes

import concourse.bass as bass
import concourse.bacc as bacc
import concourse.mybir as mybir
import concourse.tile as tile
from concourse import bass_utils

F32 = mybir.dt.float32
BF16 = mybir.dt.bfloat16

D = 1024          # model dim
S = 1024          # text sequence
SI = 512          # image sequence
SK = S + SI       # 1536 keys
HL = 8            # heads per core
DH = 64           # head dim
P = 128
KT = SK // P      # 12 key tiles
OC = HL * DH      # 512 = per-core projection output dim

_CACHE = {}


def _build_nc(analysis=False, stop_after=None, rs_chunks=4):
    nc = bacc.Bacc("TRN2", target_bir_lowering=False, debug=False, num_devices=8)

    # All inputs arrive pre-chunked on the host as [128, n_chunks, width] so
    # each tensor loads with 1-2 wide DMAs (HWDGE issue is ~625ns per DMA --
    # 72 row-tile DMAs would serialize 45us of startup).
    hT = nc.dram_tensor("hT", [P, 8, S], BF16, kind="ExternalInput")
    iT = nc.dram_tensor("iT", [P, 8, SI], BF16, kind="ExternalInput")
    mT = nc.dram_tensor("mT", [P, KT, S], BF16, kind="ExternalInput")
    # wq/wk/uk are pair-major [128, pair, k-chunk, 128] so the pair-0 slice
    # needed by the first projections is one contiguous DMA
    wq = nc.dram_tensor("wq", [P, 4, 8, P], BF16, kind="ExternalInput")
    wk = nc.dram_tensor("wk", [P, 4, 8, P], BF16, kind="ExternalInput")
    wv = nc.dram_tensor("wv", [P, 8, OC], BF16, kind="ExternalInput")
    uk = nc.dram_tensor("uk", [P, 4, 8, P], BF16, kind="ExternalInput")
    uv = nc.dram_tensor("uv", [P, 8, OC], BF16, kind="ExternalInput")
    wo = nc.dram_tensor("wo", [P, 4, D], BF16, kind="ExternalInput")
    y = nc.dram_tensor("y", [S // 2, D], BF16, kind="ExternalOutput")

    with tile.TileContext(nc) as tc:
        _body(tc, hT, iT, mT, wq, wk, wv, uk, uv, wo, y, analysis=analysis,
              stop_after=stop_after, rs_chunks=rs_chunks)
    nc.compile()
    return nc


def _body(tc, hT, iT, mT, wq, wk, wv, uk, uv, wo, y, analysis=False,
          stop_after=None, rs_chunks=4):
    nc = tc.nc
    Exp = mybir.ActivationFunctionType.Exp

    def _finish_early():
        with tc.tile_pool(name="fin", bufs=1) as fin:
            t = fin.tile([P, D], BF16, name="fint", tag="fint")
            nc.gpsimd.memset(t, 0.0)
            for mo in range(4):
                nc.sync.dma_start(y[mo * P:(mo + 1) * P, :], t)

    from collections import deque
    from contextlib import ExitStack

    with ExitStack() as ctx:
        # Persistent intermediates (live across phases).
        op = ctx.enter_context(tc.tile_pool(name="op", bufs=1))
        qT = [op.tile([P, S], BF16, name=f"qT{i}", tag=f"qT{i}") for i in range(4)]
        kTt = [op.tile([P, SK], BF16, name=f"kT{i}", tag=f"kT{i}") for i in range(4)]
        vA = [op.tile([P, HL, DH + 1], BF16, name=f"vA{i}", tag=f"vA{i}") for i in range(KT)]
        xT = [op.tile([P, S], BF16, name=f"xT{i}", tag=f"xT{i}") for i in range(4)]

        wp = ctx.enter_context(tc.tile_pool(name="wp", bufs=1))
        mp = ctx.enter_context(tc.tile_pool(name="mp", bufs=1))
        ppool = ctx.enter_context(tc.tile_pool(name="ppool", bufs=4))
        small = ctx.enter_context(tc.tile_pool(name="small", bufs=2))
        wop = ctx.enter_context(tc.tile_pool(name="wop", bufs=1))
        stg = ctx.enter_context(tc.tile_pool(name="stg", bufs=2))
        dp = ctx.enter_context(tc.tile_pool(name="dp", bufs=1, space="DRAM"))
        # One PSUM pool for the whole kernel: a 2-slot ring of [128,1024] f32
        # tiles (4 banks) shared by projections, score tiles and the output
        # projection, plus a 2-slot ring of [65,1024] x-accumulators (4 banks).
        # Sharing the "ps" ring paces projection matmuls into the exp-wait
        # gaps of the ACT-bound attention phase.
        psum = ctx.enter_context(tc.tile_pool(name="psum", bufs=1, space="PSUM"))

        def ps_tile():
            return psum.tile([P, S], F32, name="ps", tag="ps", bufs=2)

        # ------------- input loads: one wide SBUF tile per tensor -------------
        hTw = wp.tile([P, 8, S], BF16, name="hTw", tag="hTw")
        iTw = wp.tile([P, 8, SI], BF16, name="iTw", tag="iTw")
        wqw = wp.tile([P, 4, 8, P], BF16, name="wqw", tag="wqw")
        wkw = wp.tile([P, 4, 8, P], BF16, name="wkw", tag="wkw")
        wvw = wp.tile([P, 8, OC], BF16, name="wvw", tag="wvw")
        ukw = wp.tile([P, 4, 8, P], BF16, name="ukw", tag="ukw")
        uvw = wp.tile([P, 8, OC], BF16, name="uvw", tag="uvw")
        mTw = mp.tile([P, KT, S], BF16, name="mTw", tag="mTw")
        wow = wop.tile([P, 4, D], BF16, name="wow", tag="wow")
        hTs = [hTw[:, k] for k in range(8)]
        iTs = [iTw[:, k] for k in range(8)]
        wvs = [wvw[:, k] for k in range(8)]
        uvs = [uvw[:, k] for k in range(8)]
        mTs = [mTw[:, t] for t in range(KT)]
        wo_bf = [wow[:, k] for k in range(4)]

        # The cost model's HWDGE issue (~625ns/DMA) and the DMA transfer
        # stream are both serial devices, so granularity is a tradeoff:
        # ~22 medium DMAs in exact first-use order lets the first q matmul
        # start at ~3us and the rest trickle in with the stream.
        def loadc(t, dram, c0, c1):
            nc.sync.dma_start(t[:, c0:c1], dram[:, c0:c1])

        def loadw(t, dram, o0, o1):
            # weight tensors split by output-column range (pair slices)
            nc.sync.dma_start(t[:, :, o0:o1], dram[:, :, o0:o1])

        def loadh(c0, c1, q0, q1):
            nc.sync.dma_start(hTw[:, c0:c1, q0:q1], hT[:, c0:c1, q0:q1])

        # attn(p0)-critical prefix: pair-0 slices of wq/wk/uk, hT, iT.
        loadc(wqw, wq, 0, 1)
        loadh(0, 2, 0, 512)
        loadc(wkw, wk, 0, 1)
        loadh(2, 8, 0, 512)
        loadh(0, 8, 512, 1024)
        loadc(iTw, iT, 0, 8)
        loadc(ukw, uk, 0, 1)
        loadc(mTw, mT, 0, 2)
        loadc(wvw, wv, 0, 4)
        loadc(wvw, wv, 4, 8)
        loadc(uvw, uv, 0, 4)
        loadc(uvw, uv, 4, 8)
        loadc(wqw, wq, 1, 4)
        loadc(wkw, wk, 1, 4)
        loadc(ukw, uk, 1, 4)
        loadc(mTw, mT, 2, 4)
        loadc(mTw, mT, 4, 8)
        loadc(mTw, mT, 8, 12)
        loadc(wow, wo, 0, 2)
        loadc(wow, wo, 2, 4)

        # ------------- projection work units -------------
        def proj_unit(ws, rhs_tiles, copy_fn):
            # one [128,512] output tile: 8-chunk contraction into a ring slot
            ps = ps_tile()
            for k in range(8):
                nc.tensor.matmul(ps[:, 0:512], lhsT=ws[k], rhs=rhs_tiles[k],
                                 start=(k == 0), stop=(k == 7))
            copy_fn(ps[:, 0:512])

        def qk_units(pj):
            units = []
            for nq in range(2):
                sl = slice(nq * 512, (nq + 1) * 512)
                units.append(lambda nq=nq, sl=sl: proj_unit(
                    [wqw[:, pj, k] for k in range(8)], [h[:, sl] for h in hTs],
                    lambda src, sl=sl: nc.vector.tensor_copy(qT[pj][:, sl], src)))
            # gpsimd cannot touch PSUM (BIR verifier) -- kT copies go to DVE
            for nq in range(2):
                sl = slice(nq * 512, (nq + 1) * 512)
                units.append(lambda nq=nq, sl=sl: proj_unit(
                    [wkw[:, pj, k] for k in range(8)], [h[:, sl] for h in hTs],
                    lambda src, sl=sl: nc.vector.tensor_copy(kTt[pj][:, sl], src)))
            units.append(lambda: proj_unit(
                [ukw[:, pj, k] for k in range(8)], iTs,
                lambda src: nc.vector.tensor_copy(kTt[pj][:, S:SK], src)))
            return units

        def v_unit(so):
            def run():
                ps = ps_tile()
                for k in range(8):
                    if so < 8:
                        lhsT = hTs[k][:, so * P:(so + 1) * P]
                        rhs = wvs[k]
                    else:
                        lhsT = iTs[k][:, (so - 8) * P:(so - 7) * P]
                        rhs = uvs[k]
                    nc.tensor.matmul(ps[:, 0:512], lhsT=lhsT, rhs=rhs,
                                     start=(k == 0), stop=(k == 7))
                nc.vector.tensor_copy(vA[so][:, :, 0:DH],
                                      ps[:, 0:512].rearrange("p (h d) -> p h d", h=HL))
                nc.gpsimd.memset(vA[so][:, :, DH:DH + 1], 1.0)
            return run

        # Prologue: pair-0 projections + first v chunks; everything else is
        # pumped between attention key-tiles.
        for u in qk_units(0):
            u()
        for so in range(3):
            v_unit(so)()

        work = deque()
        for so in range(3, KT):
            work.append(v_unit(so))
        pair_done_mark = {}
        for pj in range(1, 4):
            for u in qk_units(pj):
                work.append(u)
            pair_done_mark[pj] = len(work)  # units required before attn(pj)

        pumped = [0]

        def pump(n=1):
            for _ in range(n):
                if work:
                    work.popleft()()
                    pumped[0] += 1

        def pump_until(target):
            while pumped[0] < target:
                pump()

        if stop_after == "p1":
            while work:
                pump()
            _finish_early()
            return

        # ------------- attention -------------
        def _emit_xmm(pj, ko, ptA, ptB, xA, xB):
            for xp, pt, hh in ((xA, ptA, 2 * pj), (xB, ptB, 2 * pj + 1)):
                for nq in range(2):
                    nc.tensor.matmul(
                        xp[:, nq * 512:(nq + 1) * 512],
                        lhsT=vA[ko][:, hh, :],
                        rhs=pt[:, nq * 512:(nq + 1) * 512],
                        start=(ko == 0), stop=(ko == KT - 1),
                    )

        for pj in range(HL // 2):
            if pj in pair_done_mark:
                pump_until(pair_done_mark[pj])
            xA = psum.tile([DH + 1, S], F32, name="xA", tag="x", bufs=2)
            xB = psum.tile([DH + 1, S], F32, name="xB", tag="x", bufs=2)
            prev = None
            for ko in range(KT):
                spA = ps_tile()
                spB = ps_tile()
                for sp, p0 in ((spA, 0), (spB, 64)):
                    for nq in range(2):
                        nc.tensor.matmul(
                            sp[:, nq * 512:(nq + 1) * 512],
                            lhsT=kTt[pj][p0:p0 + 64, ko * P:(ko + 1) * P],
                            rhs=qT[pj][p0:p0 + 64, nq * 512:(nq + 1) * 512],
                            start=True, stop=True,
                        )
                if prev is not None:
                    _emit_xmm(pj, *prev)
                ptA = ppool.tile([P, S], BF16, name="ptA", tag="ptA")
                ptB = ppool.tile([P, S], BF16, name="ptB", tag="ptB")
                nc.scalar.activation(ptA, spA, Exp, scale=0.125)
                nc.vector.tensor_mul(ptA, ptA, mTs[ko])
                nc.scalar.activation(ptB, spB, Exp, scale=0.125)
                nc.vector.tensor_mul(ptB, ptB, mTs[ko])
                pump(1)
                prev = (ko, ptA, ptB, xA, xB)
            _emit_xmm(pj, *prev)
            for xp, p0 in ((xA, 0), (xB, 64)):
                rs = small.tile([1, S], F32, name="rs", tag="rs")
                nc.vector.reciprocal(rs, xp[DH:DH + 1, :])
                rb = small.tile([64, S], F32, name="rb", tag="rb")
                nc.gpsimd.partition_broadcast(rb, rs)
                nc.vector.tensor_mul(xT[pj][p0:p0 + 64, :], xp[0:DH, :], rb)

        while work:
            pump()

        if stop_after == "attn":
            _finish_early()
            return

        # -------- output projection + chunked ReduceScatter --------
        # Chunk c holds y-rows [even-core slice c ; odd-core slice c], so RS
        # hands rank0 the even-core rows and rank1 the odd-core rows, each
        # landing at local rows [c*256:(c+1)*256].  Partial-y exchange runs
        # in bf16; the f32 output is reconstituted on device.
        NC_ = rs_chunks                  # chunks
        MPC = 8 // NC_                   # m-tiles per chunk
        RPC = MPC // 2                   # m-tiles per half per chunk
        CROWS = RPC * P                  # local output rows per chunk
        ybounce = [dp.tile([2 * CROWS, D], BF16, name=f"ybounce{c}",
                           tag=f"ybounce{c}") for c in range(NC_)]
        yout = [dp.tile([CROWS, D], BF16, name=f"yout{c}", tag=f"yout{c}")
                for c in range(NC_)]
        ytile = {}
        chunk_of = {}
        order = []
        for c in range(NC_):
            for r in range(RPC):
                chunk_of[c * RPC + r] = (c, r)
                chunk_of[4 + c * RPC + r] = (c, RPC + r)
            order += [c * RPC + r for r in range(RPC)]
            order += [4 + c * RPC + r for r in range(RPC)]

        def cast_out(i, dst, src):
            # ACT and DVE are both idle in the y phase: split each cast in
            # half across them so the PSUM ring recycles ~2x sooner
            nc.scalar.copy(dst[:, 0:512], src[:, 0:512])
            nc.vector.tensor_copy(dst[:, 512:1024], src[:, 512:1024])

        def rs_chunk(c):
            # y is bf16 (the pair-sum is computed in bf16 either way), so the
            # post-collective move is a single DRAM->DRAM DMA per chunk.
            if not analysis:
                nc.gpsimd.collective_compute(
                    "ReduceScatter",
                    mybir.AluOpType.add,
                    replica_groups=[[0, 1], [2, 3], [4, 5], [6, 7]],
                    ins=[ybounce[c].opt()],
                    outs=[yout[c].opt()],
                )
                src = yout[c]
            else:
                src = ybounce[c][0:CROWS, :]
            nc.sync.dma_start(y[c * CROWS:(c + 1) * CROWS, :], src)

        def y_mms(mo, ks, i=0):
            yps = ytile.setdefault(mo, ps_tile())
            for k in ks:
                for nq in range(2):
                    nc.tensor.matmul(
                        yps[:, nq * 512:(nq + 1) * 512],
                        lhsT=xT[k][:, mo * P:(mo + 1) * P],
                        rhs=wo_bf[k][:, nq * 512:(nq + 1) * 512],
                        start=(k == 0), stop=(k == 3),
                    )
            return yps

        # Prefill: the first two m-tiles' k=0..2 accumulation only needs
        # xT[0..2], so in the PE stream it lands inside the last pair's
        # normalize gap and keeps the PE warm through it.
        for i, mo in enumerate(order[:2]):
            y_mms(mo, (0, 1, 2), i)
        for i, mo in enumerate(order):
            c, pos = chunk_of[mo]
            yps = y_mms(mo, (3,) if mo in ytile else (0, 1, 2, 3), i)
            ysb = stg.tile([P, D], BF16, name="ysbo", tag="yrb")
            cast_out(i, ysb, yps)
            nc.sync.dma_start(ybounce[c][pos * P:(pos + 1) * P, :], ysb)
            if i % MPC == MPC - 1 and i != len(order) - 1:
                rs_chunk(i // MPC)
        rs_chunk(NC_ - 1)


def _get_nc():
    if "nc" not in _CACHE:
        _CACHE["nc"] = _build_nc()
    return _CACHE["nc"]


def make_in_maps(hidden_states, image_hidden_states, attention_mask,
                 w_q, w_k, w_v, u_k, u_v, w_o):
    bf16 = ml_dtypes.bfloat16
    hidden = np.asarray(hidden_states, dtype=np.float32)
    image = np.asarray(image_hidden_states, dtype=np.float32)
    mask = (np.asarray(attention_mask) != 0).astype(bf16)
    w_q = np.asarray(w_q, dtype=np.float32)
    w_k = np.asarray(w_k, dtype=np.float32)
    w_v = np.asarray(w_v, dtype=np.float32)
    u_k = np.asarray(u_k, dtype=np.float32)
    u_v = np.asarray(u_v, dtype=np.float32)
    w_o = np.asarray(w_o, dtype=np.float32)

    in_maps = []
    for c in range(8):
        b, hg = c // 2, c % 2
        sl = slice(hg * OC, (hg + 1) * OC)
        def chunked(x):
            # [n_chunks*128, W] -> [128, n_chunks, W]
            n = x.shape[0] // 128
            return np.ascontiguousarray(
                x.reshape(n, 128, x.shape[1]).transpose(1, 0, 2)).astype(bf16)

        def pair_major(x):
            # [8*128, 4*128] -> [128, 4, 8, 128]
            return np.ascontiguousarray(
                x.reshape(8, 128, 4, 128).transpose(1, 2, 0, 3)).astype(bf16)

        in_maps.append({
            "hT": chunked(hidden[b].T),
            "iT": chunked(image[b].T),
            "mT": chunked(np.asarray(mask[b, 0].T)),
            "wq": pair_major(w_q[sl, :].T),
            "wk": pair_major(w_k[sl, :].T),
            "wv": chunked(w_v[sl, :].T),
            "uk": pair_major(u_k[sl, :].T),
            "uv": chunked(u_v[sl, :].T),
            "wo": chunked(w_o.T[sl, :]),
        })
    return in_maps


def run(in_maps, **kwargs):
    nc = _get_nc()
    return bass_utils.run_bass_kernel_spmd(nc, in_maps, core_ids=list(range(8)),
                                           **kwargs)


def kernel(hidden_states, image_hidden_states, attention_mask,
           w_q, w_k, w_v, u_k, u_v, w_o):
    in_maps = make_in_maps(hidden_states, image_hidden_states, attention_mask,
                           w_q, w_k, w_v, u_k, u_v, w_o)
    res = run(in_maps)
    out = np.empty((4, S, D), dtype=np.float32)
    for b in range(4):
        out[b, 0:S // 2] = res.results[2 * b]["y"]
        out[b, S // 2:S] = res.results[2 * b + 1]["y"]
    return out


# revision 15
# speedup vs baseline: 1.0065x; 1.0065x over previous
"""Trainium2 Bass kernel for CustomGPT2MultiHeadAttention (B=4, S=1024, SI=512,
D=1024, 16 heads), sharded over 8 NeuronCores.

Sharding: core c handles (batch b = c//2, head-group hg = c%2 of 8 heads).
Tensor-parallel on heads for QKV/attention; after the (per-core partial)
output projection, a pairwise ReduceScatter over {2b, 2b+1} produces disjoint
sequence halves of the final output, which the host concatenates.

All inputs are cast to bf16 on the host (the device consumed bf16 for every
matmul anyway), halving input DMA bytes and removing all on-device input
casts.  Engine budget per core (cost-model):
  PE   ~150us  704 matmuls x N=512 (projections 256, scores 192, xmm 192,
               y 64) -- MM-count optimal: scores are output-element-bound,
               the xmm's M=65 (64 V dims + softmax-denominator ones row)
               cannot pack two heads into M=128 because each head streams a
               different pT.
  ACT  ~100us  96 exp tiles [128,1024] (the only op class on ACT).
  DVE   ~90us  mask multiplies (bf16 2x), qT/vA PSUM->SBUF copies,
               reciprocals, normalize muls.
  Pool  ~30us  kT copies, partition broadcasts, part of the y casts.
The attention phase alone is ACT-bound (2 exps = 2.1us per key-tile vs
1.7us of PE work), so projection matmuls for later head-pairs are emitted
*between* attention key-tiles, through the same 2-slot PSUM ring the score
tiles use: the PE fills its exp-wait gaps with projection work and stays
dense for the whole kernel.

Device-side math per core:
  qT[o,s]  = w_q[hg] @ hidden[b]^T          (bf16 matmuls, f32 PSUM accum)
  kT[o,k'] = w_k[hg] @ hidden[b]^T  ++  u_k[hg] @ image[b]^T
  v[k',o]  = (hidden[b] ++ image[b]) @ w_v/u_v[hg]^T   (natural layout)
  per head: scoresT[k',q] = kT^T-slice . qT-slice  (K=64 contraction; head
            pairs use disjoint PE row groups 0/64 -> concurrent on silicon)
  pT = exp(scoresT/8) * maskT  (exp on ACT; no max-subtraction needed:
                                scores ~ N(0,1), exp range is safe in f32)
  per head: xT_aug[65,q] += [v | 1]^T . pT   (row 64 = masked softmax sums)
            xT[d,q] = xT_aug[0:64] * (1/sums)  (partition-broadcast recip)
  y_part[s,o] = xT^T . w_o^T[d-slice]          (partial over d)
  ReduceScatter(add) over the core pair -> y half [512, 1024] per core.
"""

import numpy as np
import ml_dtypes

import concourse.bass as bass
import concourse.bacc as bacc
import concourse.mybir as mybir
import concourse.tile as tile
from concourse import bass_utils

F32 = mybir.dt.float32
BF16 = mybir.dt.bfloat16

D = 1024          # model dim
S = 1024          # text sequence
SI = 512          # image sequence
SK = S + SI       # 1536 keys
HL = 8            # heads per core
DH = 64           # head dim
P = 128
KT = SK // P      # 12 key tiles
OC = HL * DH      # 512 = per-core projection output dim

_CACHE = {}


def _build_nc(analysis=False, stop_after=None, rs_chunks=4):
    nc = bacc.Bacc("TRN2", target_bir_lowering=False, debug=False, num_devices=8)

    # All inputs arrive pre-chunked on the host as [128, n_chunks, width] so
    # each tensor loads with 1-2 wide DMAs (HWDGE issue is ~625ns per DMA --
    # 72 row-tile DMAs would serialize 45us of startup).
    hT = nc.dram_tensor("hT", [P, 8, S], BF16, kind="ExternalInput")
    iT = nc.dram_tensor("iT", [P, 8, SI], BF16, kind="ExternalInput")
    mT = nc.dram_tensor("mT", [P, KT, S], BF16, kind="ExternalInput")
    # wq/wk/uk are pair-major [128, pair, k-chunk, 128] so the pair-0 slice
    # needed by the first projections is one contiguous DMA
    wq = nc.dram_tensor("wq", [P, 4, 8, P], BF16, kind="ExternalInput")
    wk = nc.dram_tensor("wk", [P, 4, 8, P], BF16, kind="ExternalInput")
    wv = nc.dram_tensor("wv", [P, 8, OC], BF16, kind="ExternalInput")
    uk = nc.dram_tensor("uk", [P, 4, 8, P], BF16, kind="ExternalInput")
    uv = nc.dram_tensor("uv", [P, 8, OC], BF16, kind="ExternalInput")
    wo = nc.dram_tensor("wo", [P, 4, D], BF16, kind="ExternalInput")
    y = nc.dram_tensor("y", [S // 2, D], BF16, kind="ExternalOutput")

    with tile.TileContext(nc) as tc:
        _body(tc, hT, iT, mT, wq, wk, wv, uk, uv, wo, y, analysis=analysis,
              stop_after=stop_after, rs_chunks=rs_chunks)
    nc.compile()
    return nc


def _body(tc, hT, iT, mT, wq, wk, wv, uk, uv, wo, y, analysis=False,
          stop_after=None, rs_chunks=4):
    nc = tc.nc
    Exp = mybir.ActivationFunctionType.Exp

    def _finish_early():
        with tc.tile_pool(name="fin", bufs=1) as fin:
            t = fin.tile([P, D], BF16, name="fint", tag="fint")
            nc.gpsimd.memset(t, 0.0)
            for mo in range(4):
                nc.sync.dma_start(y[mo * P:(mo + 1) * P, :], t)

    from collections import deque
    from contextlib import ExitStack

    with ExitStack() as ctx:
        # Persistent intermediates (live across phases).
        op = ctx.enter_context(tc.tile_pool(name="op", bufs=1))
        qT = [op.tile([P, S], BF16, name=f"qT{i}", tag=f"qT{i}") for i in range(4)]
        kTt = [op.tile([P, SK], BF16, name=f"kT{i}", tag=f"kT{i}") for i in range(4)]
        vA = [op.tile([P, HL, DH + 1], BF16, name=f"vA{i}", tag=f"vA{i}") for i in range(KT)]
        xT = [op.tile([P, S], BF16, name=f"xT{i}", tag=f"xT{i}") for i in range(4)]

        wp = ctx.enter_context(tc.tile_pool(name="wp", bufs=1))
        mp = ctx.enter_context(tc.tile_pool(name="mp", bufs=1))
        ppool = ctx.enter_context(tc.tile_pool(name="ppool", bufs=4))
        small = ctx.enter_context(tc.tile_pool(name="small", bufs=2))
        wop = ctx.enter_context(tc.tile_pool(name="wop", bufs=1))
        stg = ctx.enter_context(tc.tile_pool(name="stg", bufs=2))
        dp = ctx.enter_context(tc.tile_pool(name="dp", bufs=1, space="DRAM"))
        # One PSUM pool for the whole kernel: a 2-slot ring of [128,1024] f32
        # tiles (4 banks) shared by projections, score tiles and the output
        # projection, plus a 2-slot ring of [65,1024] x-accumulators (4 banks).
        # Sharing the "ps" ring paces projection matmuls into the exp-wait
        # gaps of the ACT-bound attention phase.
        psum = ctx.enter_context(tc.tile_pool(name="psum", bufs=1, space="PSUM"))

        def ps_tile():
            return psum.tile([P, S], F32, name="ps", tag="ps", bufs=2)

        # ------------- input loads: one wide SBUF tile per tensor -------------
        hTw = wp.tile([P, 8, S], BF16, name="hTw", tag="hTw")
        iTw = wp.tile([P, 8, SI], BF16, name="iTw", tag="iTw")
        wqw = wp.tile([P, 4, 8, P], BF16, name="wqw", tag="wqw")
        wkw = wp.tile([P, 4, 8, P], BF16, name="wkw", tag="wkw")
        wvw = wp.tile([P, 8, OC], BF16, name="wvw", tag="wvw")
        ukw = wp.tile([P, 4, 8, P], BF16, name="ukw", tag="ukw")
        uvw = wp.tile([P, 8, OC], BF16, name="uvw", tag="uvw")
        mTw = mp.tile([P, KT, S], BF16, name="mTw", tag="mTw")
        wow = wop.tile([P, 4, D], BF16, name="wow", tag="wow")
        hTs = [hTw[:, k] for k in range(8)]
        iTs = [iTw[:, k] for k in range(8)]
        wvs = [wvw[:, k] for k in range(8)]
        uvs = [uvw[:, k] for k in range(8)]
        mTs = [mTw[:, t] for t in range(KT)]
        wo_bf = [wow[:, k] for k in range(4)]

        # The cost model's HWDGE issue (~625ns/DMA) and the DMA transfer
        # stream are both serial devices, so granularity is a tradeoff:
        # ~22 medium DMAs in exact first-use order lets the first q matmul
        # start at ~3us and the rest trickle in with the stream.
        def loadc(t, dram, c0, c1):
            nc.sync.dma_start(t[:, c0:c1], dram[:, c0:c1])

        def loadw(t, dram, o0, o1):
            # weight tensors split by output-column range (pair slices)
            nc.sync.dma_start(t[:, :, o0:o1], dram[:, :, o0:o1])

        def loadh(c0, c1, q0, q1):
            nc.sync.dma_start(hTw[:, c0:c1, q0:q1], hT[:, c0:c1, q0:q1])

        # attn(p0)-critical prefix: pair-0 slices of wq/wk/uk, hT, iT.
        loadc(wqw, wq, 0, 1)
        loadh(0, 2, 0, 512)
        loadc(wkw, wk, 0, 1)
        loadh(2, 8, 0, 512)
        loadh(0, 8, 512, 1024)
        loadc(iTw, iT, 0, 8)
        loadc(ukw, uk, 0, 1)
        loadc(mTw, mT, 0, 2)
        loadc(wvw, wv, 0, 4)
        loadc(wvw, wv, 4, 8)
        loadc(uvw, uv, 0, 4)
        loadc(uvw, uv, 4, 8)
        loadc(wqw, wq, 1, 4)
        loadc(wkw, wk, 1, 4)
        loadc(ukw, uk, 1, 4)
        loadc(mTw, mT, 2, 4)
        loadc(mTw, mT, 4, 8)
        loadc(mTw, mT, 8, 12)
        loadc(wow, wo, 0, 2)
        loadc(wow, wo, 2, 4)

        # ------------- projection work units -------------
        def proj_unit(ws, rhs_tiles, copy_fn):
            # one [128,512] output tile: 8-chunk contraction into a ring slot
            ps = ps_tile()
            for k in range(8):
                nc.tensor.matmul(ps[:, 0:512], lhsT=ws[k], rhs=rhs_tiles[k],
                                 start=(k == 0), stop=(k == 7))
            copy_fn(ps[:, 0:512])

        def qk_units(pj):
            units = []
            for nq in range(2):
                sl = slice(nq * 512, (nq + 1) * 512)
                units.append(lambda nq=nq, sl=sl: proj_unit(
                    [wqw[:, pj, k] for k in range(8)], [h[:, sl] for h in hTs],
                    lambda src, sl=sl: nc.vector.tensor_copy(qT[pj][:, sl], src)))
            # gpsimd cannot touch PSUM (BIR verifier) -- kT copies go to DVE
            for nq in range(2):
                sl = slice(nq * 512, (nq + 1) * 512)
                units.append(lambda nq=nq, sl=sl: proj_unit(
                    [wkw[:, pj, k] for k in range(8)], [h[:, sl] for h in hTs],
                    lambda src, sl=sl: nc.vector.tensor_copy(kTt[pj][:, sl], src)))
            units.append(lambda: proj_unit(
                [ukw[:, pj, k] for k in range(8)], iTs,
                lambda src: nc.vector.tensor_copy(kTt[pj][:, S:SK], src)))
            return units

        def v_unit(so):
            def run():
                ps = ps_tile()
                for k in range(8):
                    if so < 8:
                        lhsT = hTs[k][:, so * P:(so + 1) * P]
                        rhs = wvs[k]
                    else:
                        lhsT = iTs[k][:, (so - 8) * P:(so - 7) * P]
                        rhs = uvs[k]
                    nc.tensor.matmul(ps[:, 0:512], lhsT=lhsT, rhs=rhs,
                                     start=(k == 0), stop=(k == 7))
                nc.vector.tensor_copy(vA[so][:, :, 0:DH],
                                      ps[:, 0:512].rearrange("p (h d) -> p h d", h=HL))
                nc.gpsimd.memset(vA[so][:, :, DH:DH + 1], 1.0)
            return run

        # Prologue: pair-0 projections + first v chunks; everything else is
        # pumped between attention key-tiles.
        for u in qk_units(0):
            u()
        for so in range(3):
            v_unit(so)()

        work = deque()
        for so in range(3, KT):
            work.append(v_unit(so))
        pair_done_mark = {}
        for pj in range(1, 4):
            for u in qk_units(pj):
                work.append(u)
            pair_done_mark[pj] = len(work)  # units required before attn(pj)

        pumped = [0]

        def pump(n=1):
            for _ in range(n):
                if work:
                    work.popleft()()
                    pumped[0] += 1

        def pump_until(target):
            while pumped[0] < target:
                pump()

        if stop_after == "p1":
            while work:
                pump()
            _finish_early()
            return

        # ------------- attention -------------
        def _emit_xmm(pj, ko, ptA, ptB, xA, xB):
            for xp, pt, hh in ((xA, ptA, 2 * pj), (xB, ptB, 2 * pj + 1)):
                for nq in range(2):
                    nc.tensor.matmul(
                        xp[:, nq * 512:(nq + 1) * 512],
                        lhsT=vA[ko][:, hh, :],
                        rhs=pt[:, nq * 512:(nq + 1) * 512],
                        start=(ko == 0), stop=(ko == KT - 1),
                    )

        for pj in range(HL // 2):
            if pj in pair_done_mark:
                pump_until(pair_done_mark[pj])
            xA = psum.tile([DH + 1, S], F32, name="xA", tag="x", bufs=2)
            xB = psum.tile([DH + 1, S], F32, name="xB", tag="x", bufs=2)
            prev = None
            for ko in range(KT):
                spA = ps_tile()
                spB = ps_tile()
                for sp, p0 in ((spA, 0), (spB, 64)):
                    for nq in range(2):
                        nc.tensor.matmul(
                            sp[:, nq * 512:(nq + 1) * 512],
                            lhsT=kTt[pj][p0:p0 + 64, ko * P:(ko + 1) * P],
                            rhs=qT[pj][p0:p0 + 64, nq * 512:(nq + 1) * 512],
                            start=True, stop=True,
                        )
                if prev is not None:
                    _emit_xmm(pj, *prev)
                ptA = ppool.tile([P, S], BF16, name="ptA", tag="ptA")
                ptB = ppool.tile([P, S], BF16, name="ptB", tag="ptB")
                nc.scalar.activation(ptA, spA, Exp, scale=0.125)
                nc.vector.tensor_mul(ptA, ptA, mTs[ko])
                nc.scalar.activation(ptB, spB, Exp, scale=0.125)
                nc.vector.tensor_mul(ptB, ptB, mTs[ko])
                pump(1)
                prev = (ko, ptA, ptB, xA, xB)
            _emit_xmm(pj, *prev)
            for xp, p0 in ((xA, 0), (xB, 64)):
                rs = small.tile([1, S], F32, name="rs", tag="rs")
                nc.vector.reciprocal(rs, xp[DH:DH + 1, :])
                rb = small.tile([64, S], F32, name="rb", tag="rb")
                nc.gpsimd.partition_broadcast(rb, rs)
                nc.vector.tensor_mul(xT[pj][p0:p0 + 64, :], xp[0:DH, :], rb)

        while work:
            pump()

        if stop_after == "attn":
            _finish_early()
            return

        # -------- output projection + chunked ReduceScatter --------
        # Chunk c holds y-rows [even-core slice c ; odd-core slice c], so RS
        # hands rank0 the even-core rows and rank1 the odd-core rows, each
        # landing at local rows [c*256:(c+1)*256].  Partial-y exchange runs
        # in bf16; the f32 output is reconstituted on device.
        NC_ = rs_chunks                  # chunks
        MPC = 8 // NC_                   # m-tiles per chunk
        RPC = MPC // 2                   # m-tiles per half per chunk
        CROWS = RPC * P                  # local output rows per chunk
        ybounce = [dp.tile([2 * CROWS, D], BF16, name=f"ybounce{c}",
                           tag=f"ybounce{c}") for c in range(NC_)]
        yout = [dp.tile([CROWS, D], BF16, name=f"yout{c}", tag=f"yout{c}")
                for c in range(NC_)]
        ytile = {}
        chunk_of = {}
        order = []
        for c in range(NC_):
            for r in range(RPC):
                chunk_of[c * RPC + r] = (c, r)
                chunk_of[4 + c * RPC + r] = (c, RPC + r)
            order += [c * RPC + r for r in range(RPC)]
            order += [4 + c * RPC + r for r in range(RPC)]

        def cast_out(i, dst, src):
            # ACT and DVE are idle in the y phase; Pool cannot touch PSUM
            eng = (nc.scalar.copy, nc.vector.tensor_copy)[i % 2]
            eng(dst, src)

        def rs_chunk(c):
            # y is bf16 (the pair-sum is computed in bf16 either way), so the
            # post-collective move is a single DRAM->DRAM DMA per chunk.
            if not analysis:
                nc.gpsimd.collective_compute(
                    "ReduceScatter",
                    mybir.AluOpType.add,
                    replica_groups=[[0, 1], [2, 3], [4, 5], [6, 7]],
                    ins=[ybounce[c].opt()],
                    outs=[yout[c].opt()],
                )
                src = yout[c]
            else:
                src = ybounce[c][0:CROWS, :]
            nc.sync.dma_start(y[c * CROWS:(c + 1) * CROWS, :], src)

        def y_mms(mo, ks, i=0):
            yps = ytile.setdefault(mo, ps_tile())
            for k in ks:
                for nq in range(2):
                    nc.tensor.matmul(
                        yps[:, nq * 512:(nq + 1) * 512],
                        lhsT=xT[k][:, mo * P:(mo + 1) * P],
                        rhs=wo_bf[k][:, nq * 512:(nq + 1) * 512],
                        start=(k == 0), stop=(k == 3),
                    )
            return yps

        # Prefill: the first two m-tiles' k=0..2 accumulation only needs
        # xT[0..2], so in the PE stream it lands inside the last pair's
        # normalize gap and keeps the PE warm through it.
        for i, mo in enumerate(order[:2]):
            y_mms(mo, (0, 1, 2), i)
        for i, mo in enumerate(order):
            c, pos = chunk_of[mo]
            yps = y_mms(mo, (3,) if mo in ytile else (0, 1, 2, 3), i)
            ysb = stg.tile([P, D], BF16, name="ysbo", tag="yrb")
            cast_out(i, ysb, yps)
            nc.sync.dma_start(ybounce[c][pos * P:(pos + 1) * P, :], ysb)
            if i % MPC == MPC - 1 and i != len(order) - 1:
                rs_chunk(i // MPC)
        rs_chunk(NC_ - 1)


def _get_nc():
    if "nc" not in _CACHE:
        _CACHE["nc"] = _build_nc()
    return _CACHE["nc"]


def make_in_maps(hidden_states, image_hidden_states, attention_mask,
                 w_q, w_k, w_v, u_k, u_v, w_o):
    bf16 = ml_dtypes.bfloat16
    hidden = np.asarray(hidden_states, dtype=np.float32)
    image = np.asarray(image_hidden_states, dtype=np.float32)
    mask = (np.asarray(attention_mask) != 0).astype(bf16)
    w_q = np.asarray(w_q, dtype=np.float32)
    w_k = np.asarray(w_k, dtype=np.float32)
    w_v = np.asarray(w_v, dtype=np.float32)
    u_k = np.asarray(u_k, dtype=np.float32)
    u_v = np.asarray(u_v, dtype=np.float32)
    w_o = np.asarray(w_o, dtype=np.float32)

    in_maps = []
    for c in range(8):
        b, hg = c // 2, c % 2
        sl = slice(hg * OC, (hg + 1) * OC)
        def chunked(x):
            # [n_chunks*128, W] -> [128, n_chunks, W]
            n = x.shape[0] // 128
            return np.ascontiguousarray(
                x.reshape(n, 128, x.shape[1]).transpose(1, 0, 2)).astype(bf16)

        def pair_major(x):
            # [8*128, 4*128] -> [128, 4, 8, 128]
            return np.ascontiguousarray(
                x.reshape(8, 128, 4, 128).transpose(1, 2, 0, 3)).astype(bf16)

        in_maps.append({
            "hT": chunked(hidden[b].T),
            "iT": chunked(image[b].T),
            "mT": chunked(np.asarray(mask[b, 0].T)),
            "wq": pair_major(w_q[sl, :].T),
            "wk": pair_major(w_k[sl, :].T),
            "wv": chunked(w_v[sl, :].T),
            "uk": pair_major(u_k[sl, :].T),
            "uv": chunked(u_v[sl, :].T),
            "wo": chunked(w_o.T[sl, :]),
        })
    return in_maps


def run(in_maps, **kwargs):
    nc = _get_nc()
    return bass_utils.run_bass_kernel_spmd(nc, in_maps, core_ids=list(range(8)),
                                           **kwargs)


def kernel(hidden_states, image_hidden_states, attention_mask,
           w_q, w_k, w_v, u_k, u_v, w_o):
    in_maps = make_in_maps(hidden_states, image_hidden_states, attention_mask,
                           w_q, w_k, w_v, u_k, u_v, w_o)
    res = run(in_maps)
    out = np.empty((4, S, D), dtype=np.float32)
    for b in range(4):
        out[b, 0:S // 2] = res.results[2 * b]["y"]
        out[b, S // 2:S] = res.results[2 * b + 1]["y"]
    return out
